# revision 11
# baseline (speedup 1.0000x reference)
"""Distributed Trainium2 kernel for nn_AlternateConvolution (node_layer branch).

Reference computation:
    d      = (H_e @ p.T)[:, 0]                    # [N_E] per-edge scalar
    G      = (T * d[None, :]) @ T.T               # [N_V, N_V]  (symmetric!)
    M1     = eye + (1 - eye) * G                  # diagonal forced to 1
    A      = M1 * adj_v
    ret    = A @ (H_v @ weight) + bias            # [N_V, OUT_V]
    return (ret, H_e)

Distribution (8 NeuronCores, SPMD single NEFF), v4 — balanced symmetry:
    Core c owns output rows R_c = [512c, 512(c+1)).  G is symmetric, so only
    node-block pairs at ring distance 0..4 are computed: core c computes
    G[j-block (c+g)%8, R_c] for g in {0..3} in full, and HALF of the
    distance-4 block — the two ends of each distance-4 pair compute disjoint
    halves of the e-contraction (a per-core host-side permutation of the
    e axis keeps the SPMD graph uniform) and exchange partials.  Every core
    thus does exactly 4.5 block-equivalents of matmul work (36/64 of naive).

    Blocks at distance 1..3 plus the distance-4 partial are exchanged via one
    AllGather; each core picks the tiles it needs with a partition-id-derived
    dynamic DMA offset and DMA-transposes them (G[j,i] = G[i,j], also true
    per e-subset).  The output GEMM accumulates transposed
    (ret.T[o,i] += W2[j,o].T-stationary @ A.T[j,i]-moving, one PSUM bank,
    N=512 per step); ret.T is PE-transposed back at the end.

    All per-core block indexing is pre-rotated host-side (TTt4/TTH/ADJt/HVTR
    are in ring order (c+g)%8) so all 8 cores run one identical graph.  The
    "diagonal forced to 1" term is applied as ret[i] += adj_v[i,i] * W2[i]
    at the end (adj diagonal zeroed host-side).

    Operands fp16 (host-cast); accumulation fp32 in PSUM.
"""

import numpy as np

N_V, N_E = 4096, 16384
IN_V, OUT_V, IN_E = 128, 128, 64
NCORES = 8
RB = N_V // NCORES          # 512 output rows per core
ETB = 16                    # e superblocks (1024 edges each)
EL = 8                      # e chunks (128 edges) per superblock
E_T = ETB * EL              # 128 e chunks total
G_BLK = 8                   # node blocks of 512
JBL = 4                     # j chunks (128) per j block
TILE_ELEMS = JBL * 128 * 512  # elements per exchanged [512, 512] tile

_F16 = np.float16
_cache = {}


def _build():
    import concourse.mybir as mybir
    from concourse import bacc
    from concourse.masks import make_identity
    from concourse.tile import TileContext

    dt = mybir.dt
    f16, f32 = dt.float16, dt.float32

    nc = bacc.Bacc("TRN2", target_bir_lowering=False, debug=False,
                   num_devices=NCORES)

    # ------- DRAM parameters (host-pretiled, per-core ring-rotated) ------
    TTt4 = nc.dram_tensor("TTt4", [4, ETB, 128, EL * 512], f16,
                          kind="ExternalInput")
    TTH = nc.dram_tensor("TTH", [ETB // 2, 128, EL * 512], f16,
                         kind="ExternalInput")
    TTB = nc.dram_tensor("TTB", [ETB, 128, EL * 512], f16, kind="ExternalInput")
    ADJt = nc.dram_tensor("ADJt", [G_BLK, 128, JBL * 512], f16,
                          kind="ExternalInput")
    HVTR = nc.dram_tensor("HVTR", [IN_V, N_V], f16, kind="ExternalInput")
    HVTB = nc.dram_tensor("HVTB", [IN_V, RB], f16, kind="ExternalInput")
    WT = nc.dram_tensor("WT", [IN_V, OUT_V], f16, kind="ExternalInput")
    HET = nc.dram_tensor("HET", [IN_E, N_E], f16, kind="ExternalInput")
    PC = nc.dram_tensor("PC", [IN_E, 1], f16, kind="ExternalInput")
    ADIAB = nc.dram_tensor("ADIAB", [128, RB], f32, kind="ExternalInput")
    BIAC = nc.dram_tensor("BIAC", [128, 1], f32, kind="ExternalInput")
    OUT = nc.dram_tensor("out", [RB, OUT_V], f32, kind="ExternalOutput")

    with TileContext(nc) as tc:
        const = tc.alloc_tile_pool(name="const", bufs=1)
        spool = tc.alloc_tile_pool(name="spool", bufs=1)
        dramp = tc.alloc_tile_pool(name="dramp", bufs=1, space="DRAM")
        hetp = tc.alloc_tile_pool(name="hetp", bufs=1)
        stage = tc.alloc_tile_pool(name="stage", bufs=2)
        psum0 = tc.alloc_tile_pool(name="psum0", bufs=2, space="PSUM")

        # long-lived
        wt_sb = const.tile([IN_V, OUT_V], f16)
        hvtb_sb = const.tile([IN_V, RB], f16)
        adiab_sb = const.tile([128, RB], f32)
        biac_sb = const.tile([128, 1], f32)
        ident_sb = const.tile([128, 128], f32)
        w2_sb = const.tile([128, N_V], f16)          # H_v @ weight (ring order)
        s_sb = spool.tile([128, E_T * 512], f16)     # scaled T.T[:, R_c]
        # prologue-only (pool released before the main loop).  Order matters:
        # later pools reuse this range bottom-up, so put the tiles whose last
        # read happens earliest (het: d-matmuls) at the bottom where the tt
        # stream lands, and the latest-read (d: S-scaling) higher up.
        het_sb = hetp.tile([IN_E, N_E], f16)
        hvt_sb = hetp.tile([IN_V, N_V], f16)
        d_sb = hetp.tile([128, E_T], f32)
        pc_sb = hetp.tile([IN_E, 1], f16)

        contrib = dramp.tile([4, JBL, 128, 512], f16, name="contrib")
        agout = dramp.tile([NCORES, 4 * TILE_ELEMS], f16, name="agout",
                           addr_space="Shared")
        exst = dramp.tile([4, TILE_ELEMS], f16, name="exst")

        # HET lands first, chunked, so the d -> S-scale -> main-loop chain
        # starts as early as possible (it is the kernel's serial prefix)
        nc.sync.dma_start(out=pc_sb[:], in_=PC[:])
        for etb in range(ETB):
            nc.sync.dma_start(out=het_sb[:, etb * EL * 128:(etb + 1) * EL * 128],
                              in_=HET[:, etb * EL * 128:(etb + 1) * EL * 128])
        nc.sync.dma_start(out=wt_sb[:], in_=WT[:])
        nc.sync.dma_start(out=hvt_sb[:], in_=HVTR[:])
        nc.sync.dma_start(out=hvtb_sb[:], in_=HVTB[:])
        nc.sync.dma_start(out=adiab_sb[:], in_=ADIAB[:])
        nc.sync.dma_start(out=biac_sb[:], in_=BIAC[:])
        make_identity(nc, ident_sb[:])

        # d[e] = (H_e @ p.T): one PSUM column per 128-edge chunk, copied out
        # per-superblock so S scaling (and the PE main loop) starts early
        d_ps = psum0.tile([128, E_T], f32, tag="dps")
        for etb in range(ETB):
            for el in range(EL):
                et = etb * EL + el
                nc.tensor.matmul(d_ps[:, et:et + 1],
                                 het_sb[:, et * 128:(et + 1) * 128], pc_sb[:],
                                 start=True, stop=True)
            nc.vector.tensor_copy(d_sb[:, etb * EL:(etb + 1) * EL],
                                  d_ps[:, etb * EL:(etb + 1) * EL])

        # W2 = H_v @ weight in ring order ([j-partition, o] tiles)
        for jb in range(N_V // 128):
            w_ps = psum0.tile([128, OUT_V], f32, tag="wps")
            nc.tensor.matmul(w_ps[:], hvt_sb[:, jb * 128:(jb + 1) * 128],
                             wt_sb[:], start=True, stop=True)
            nc.vector.tensor_copy(w2_sb[:, jb * 128:(jb + 1) * 128], w_ps[:])

        # S = T.T[:, R_c] * d[e]  (per-partition scale), resident in SBUF
        for etb in range(ETB):
            sraw = stage.tile([128, EL * 512], f16, tag="sraw")
            nc.sync.dma_start(out=sraw[:], in_=TTB[etb])
            for el in range(EL):
                et = etb * EL + el
                nc.vector.tensor_scalar_mul(
                    s_sb[:, et * 512:(et + 1) * 512],
                    sraw[:, el * 512:(el + 1) * 512],
                    d_sb[:, et:et + 1])

        psum0.release()
        stage.release()
        hetp.release()

        ttp = tc.alloc_tile_pool(name="ttp", bufs=3)
        adjp = tc.alloc_tile_pool(name="adjp", bufs=2)
        atp = tc.alloc_tile_pool(name="atp", bufs=4)
        gexp = tc.alloc_tile_pool(name="gexp", bufs=2)
        g4p = tc.alloc_tile_pool(name="g4p", bufs=1)
        rtp = tc.alloc_tile_pool(name="rtp", bufs=4)
        rxat = tc.alloc_tile_pool(name="rxat", bufs=1)
        outp = tc.alloc_tile_pool(name="outp", bufs=1)
        psum_m = tc.alloc_tile_pool(name="psum_m", bufs=5, space="PSUM")
        psum_r = tc.alloc_tile_pool(name="psum_r", bufs=1, space="PSUM")

        retT_ps = psum_r.tile([128, RB], f32, name="retT")  # [o, i], 1 bank
        n_ret = G_BLK * JBL
        ret_idx = 0

        def ret_mm(at, jb):
            # ret.T[o, i] += W2[jb-chunk, o].T @ A.T[jb-chunk, i]
            nonlocal ret_idx
            nc.tensor.matmul(retT_ps[:], w2_sb[:, jb * 128:(jb + 1) * 128],
                             at[:], start=(ret_idx == 0),
                             stop=(ret_idx == n_ret - 1))
            ret_idx += 1

        # own half of the distance-4 block, kept for the post-exchange sum
        gex4 = [g4p.tile([128, 512], f16, tag=f"g4_{j}", name=f"g4_{j}")
                for j in range(JBL)]

        # ---- main loop, part 1: blocks at ring distance 1,2,3 + d4-half ----
        def block(g):
            half = (g == 4)
            n_etb = ETB // 2 if half else ETB
            last_et = n_etb * EL - 1
            adj_g = None
            if not half:
                adj_g = adjp.tile([128, JBL * 512], f16, tag="adjg",
                                  name=f"adj_{g}")
                nc.scalar.dma_start(out=adj_g[:], in_=ADJt[g])
            m_ps = [psum_m.tile([128, 512], f32, tag="mps", name=f"m_{g}_{j}")
                    for j in range(JBL)]
            for etb in range(n_etb):
                tt = ttp.tile([128, EL * 512], f16, tag="tt",
                              name=f"tt_{g}_{etb}")
                nc.scalar.dma_start(out=tt[:],
                                     in_=TTH[etb] if half else TTt4[g, etb])
                for el in range(EL):
                    et = etb * EL + el
                    s_slice = s_sb[:, et * 512:(et + 1) * 512]
                    for jbl in range(JBL):
                        nc.tensor.matmul(
                            m_ps[jbl][:],
                            tt[:, el * 512 + jbl * 128:el * 512 + (jbl + 1) * 128],
                            s_slice,
                            start=(et == 0), stop=(et == last_et))
            for jbl in range(JBL):
                jb = g * JBL + jbl
                if half:
                    nc.vector.tensor_copy(gex4[jbl][:], m_ps[jbl][:])
                    nc.sync.dma_start(out=contrib[3, jbl], in_=gex4[jbl][:])
                    continue
                at = atp.tile([128, 512], f16, tag="at", name=f"at_{jb}")
                if g in (1, 2, 3):
                    gex = gexp.tile([128, 512], f16, tag="gex",
                                    name=f"gex_{jb}")
                    nc.vector.tensor_copy(gex[:], m_ps[jbl][:])
                    nc.sync.dma_start(out=contrib[g - 1, jbl], in_=gex[:])
                    nc.vector.tensor_mul(at[:], gex[:],
                                         adj_g[:, jbl * 512:(jbl + 1) * 512])
                else:
                    nc.vector.tensor_mul(at[:], m_ps[jbl][:],
                                         adj_g[:, jbl * 512:(jbl + 1) * 512])
                ret_mm(at, jb)

        for g in (1, 2, 3, 4):
            block(g)

        # all four exchange tiles staged -> gather across the chip
        nc.gpsimd.collective_compute(
            "AllGather", mybir.AluOpType.bypass,
            replica_groups=[list(range(NCORES))],
            ins=[contrib[:].opt()],
            outs=[agout[:].opt()])

        # hoist the gathers + transposes so they overlap the g=0 block.
        # distance d in 5..7: need transpose of core (c+d)%8's distance-(8-d)
        # tile (slot 7-d); distance 4: partner's half-partial (slot 3).
        rts = {}
        for d in (4, 5, 6, 7):
            src = nc.gpsimd.alloc_register(f"src{d}")
            pid = nc.gpsimd.partition_id()
            nc.gpsimd.reg_add(src, pid, d)
            nc.gpsimd.reg_mod(src, src, NCORES)
            srcv = nc.gpsimd.snap(src, donate=True, min_val=0,
                                  max_val=NCORES - 1)
            slot = 3 if d == 4 else 7 - d
            nc.gpsimd.dma_start(
                out=exst[d - 4:d - 3, :],
                in_=agout[_ds(srcv, 1),
                          slot * TILE_ELEMS:(slot + 1) * TILE_ELEMS])
        exr = exst[:].rearrange("a (r c) -> a r c", r=512)
        # d = 4, 5: transpose + blend hoisted so they overlap the g=0 block
        for d in (4, 5):
            adj_g = adjp.tile([128, JBL * 512], f16, tag="adjg",
                              name=f"adj_{d}")
            nc.sync.dma_start(out=adj_g[:], in_=ADJt[d])
            for jbl in range(JBL):
                jb = d * JBL + jbl
                rt = rtp.tile([128, 512], f16, tag="rt", name=f"rt_{jb}")
                nc.sync.dma_start_transpose(
                    rt[:], exr[d - 4, :, jbl * 128:(jbl + 1) * 128])
                at = rxat.tile([128, 512], f16, tag=f"rx_{jb}",
                               name=f"rx_{jb}")
                if d == 4:
                    nc.vector.tensor_add(at[:], rt[:], gex4[jbl][:])
                    nc.vector.tensor_mul(at[:], at[:],
                                         adj_g[:, jbl * 512:(jbl + 1) * 512])
                else:
                    nc.vector.tensor_mul(at[:], rt[:],
                                         adj_g[:, jbl * 512:(jbl + 1) * 512])
                rts[jb] = at

        # ---- main loop, part 2: the local distance-0 block ----
        block(0)

        # ---- fold the exchanged tiles into the output GEMM ----
        for d in (4, 5):
            for jbl in range(JBL):
                jb = d * JBL + jbl
                ret_mm(rts[jb], jb)
        for d in (6, 7):
            adj_g = adjp.tile([128, JBL * 512], f16, tag="adjg",
                              name=f"adj_{d}")
            nc.sync.dma_start(out=adj_g[:], in_=ADJt[d])
            for jbl in range(JBL):
                jb = d * JBL + jbl
                rt = rtp.tile([128, 512], f16, tag="rt", name=f"rt_{jb}")
                nc.sync.dma_start_transpose(
                    rt[:], exr[d - 4, :, jbl * 128:(jbl + 1) * 128])
                at = atp.tile([128, 512], f16, tag="at", name=f"at_{jb}")
                nc.vector.tensor_mul(at[:], rt[:],
                                     adj_g[:, jbl * 512:(jbl + 1) * 512])
                ret_mm(at, jb)
        assert ret_idx == n_ret

        # ---- epilogue ----
        psum_e = tc.alloc_tile_pool(name="psum_e", bufs=1, space="PSUM")
        # W2[R_c].T[o, i] for the diagonal correction
        w2bt_ps = psum_e.tile([128, RB], f32, tag="w2bt")
        nc.tensor.matmul(w2bt_ps[:], wt_sb[:], hvtb_sb[:],
                         start=True, stop=True)
        corr = outp.tile([128, RB], f32, tag="corr", name="corr")
        nc.vector.tensor_mul(corr[:], w2bt_ps[:], adiab_sb[:])
        sum1 = outp.tile([128, RB], f32, tag="sum1", name="sum1")
        nc.vector.tensor_add(sum1[:], retT_ps[:], corr[:])
        res = outp.tile([128, RB], f32, tag="res", name="res")
        nc.vector.tensor_scalar_add(res[:], sum1[:], biac_sb[:, 0:1])
        # transpose [o, i] -> [i, o] and store
        for ic in range(RB // 128):
            t_ps = psum_e.tile([128, 128], f32, tag="tps", name=f"t_{ic}")
            nc.tensor.transpose(t_ps[:], res[:, ic * 128:(ic + 1) * 128],
                                ident_sb[:])
            o_sb = outp.tile([128, OUT_V], f32, tag="osb", name=f"o_{ic}")
            nc.vector.tensor_copy(o_sb[:], t_ps[:])
            nc.sync.dma_start(out=OUT[ic * 128:(ic + 1) * 128, :], in_=o_sb[:])

        psum_e.release()
        psum_r.release()
        psum_m.release()
        outp.release()
        rxat.release()
        rtp.release()
        g4p.release()
        gexp.release()
        atp.release()
        adjp.release()
        ttp.release()
        dramp.release()
        spool.release()
        const.release()

    nc.compile()
    return nc


def _ds(start, size):
    import concourse.bass as bass
    return bass.ds(start, size)


def _prep_inputs(H_v, H_e, adj_v, T, weight, p, bias):
    """Host-side shard/retile/rotate/e-permute. Returns in_maps for 8 cores."""
    TT16 = np.ascontiguousarray(T.T).astype(_F16)          # [N_E, N_V]
    A = TT16.reshape(ETB, EL, 128, G_BLK, 512)
    TTt_all = np.ascontiguousarray(A.transpose(3, 0, 2, 1, 4)).reshape(
        G_BLK, ETB, 128, EL * 512)                          # [b, etb, p, el*j]

    HVT = np.ascontiguousarray(H_v.T).astype(_F16)         # [IN_V, N_V]
    WT = weight.astype(_F16)
    HET_n = np.ascontiguousarray(H_e.T).astype(_F16)       # [IN_E, N_E]
    PCm = np.ascontiguousarray(p.T).astype(_F16)           # [IN_E, 1]
    BIAC = np.ascontiguousarray(bias.astype(np.float32)[:, None])
    adjT = adj_v.T                                          # [j, i] view
    diag = np.ascontiguousarray(np.diag(adj_v)).astype(np.float32)

    in_maps = []
    for c in range(NCORES):
        r0, r1 = c * RB, (c + 1) * RB
        ring = [(c + g) % G_BLK for g in range(G_BLK)]
        if c < NCORES // 2:
            eperm = list(range(ETB))
        else:
            eperm = list(range(ETB // 2, ETB)) + list(range(ETB // 2))
        TTb = TT16[:, r0:r1]                                # [N_E, RB]
        B = TTb.reshape(ETB, EL, 128, 512)
        TTB = np.ascontiguousarray(
            B.transpose(0, 2, 1, 3)[eperm]).reshape(ETB, 128, EL * 512)
        HET = np.ascontiguousarray(
            HET_n.reshape(IN_E, ETB, EL * 128)[:, eperm]).reshape(IN_E, N_E)
        adjblk = adjT[:, r0:r1].astype(_F16)                # [N_V, RB]
        adjblk[r0 + np.arange(RB), np.arange(RB)] = 0       # zero the diagonal
        C = adjblk.reshape(G_BLK, JBL, 128, 512).transpose(0, 2, 1, 3)
        ADJt = np.ascontiguousarray(C[ring]).reshape(G_BLK, 128, JBL * 512)
        cols = np.concatenate([np.arange(b * RB, (b + 1) * RB) for b in ring])
        HVTR = np.ascontiguousarray(HVT[:, cols])
        ADIAB = np.ascontiguousarray(
            np.broadcast_to(diag[r0:r1], (128, RB)))
        ttg = TTt_all[:, eperm]                             # e-permuted blocks
        in_maps.append({
            "TTt4": np.ascontiguousarray(ttg[ring[:4]]),
            "TTH": np.ascontiguousarray(ttg[ring[4], :ETB // 2]),
            "TTB": TTB,
            "ADJt": ADJt,
            "HVTR": HVTR,
            "HVTB": np.ascontiguousarray(HVT[:, r0:r1]),
            "WT": WT,
            "HET": HET,
            "PC": PCm,
            "ADIAB": ADIAB,
            "BIAC": BIAC,
        })
    return in_maps


def kernel(H_v, H_e, adj_e, adj_v, T, weight, p, bias):
    from concourse.bass_utils import run_bass_kernel_spmd

    H_v = np.asarray(H_v, dtype=np.float32)
    H_e = np.asarray(H_e, dtype=np.float32)
    adj_v = np.asarray(adj_v, dtype=np.float32)
    T = np.asarray(T, dtype=np.float32)
    weight = np.asarray(weight, dtype=np.float32)
    p = np.asarray(p, dtype=np.float32)
    bias = np.asarray(bias, dtype=np.float32)

    if "nc" not in _cache:
        _cache["nc"] = _build()
    nc = _cache["nc"]

    in_maps = _prep_inputs(H_v, H_e, adj_v, T, weight, p, bias)
    res = run_bass_kernel_spmd(nc, in_maps, list(range(NCORES)))
    ret = np.concatenate([res.results[c]["out"] for c in range(NCORES)], axis=0)
    return (ret, H_e)


# revision 12
# speedup vs baseline: 1.0293x; 1.0293x over previous
"""Distributed Trainium2 kernel for nn_AlternateConvolution (node_layer branch).

Reference computation:
    d      = (H_e @ p.T)[:, 0]                    # [N_E] per-edge scalar
    G      = (T * d[None, :]) @ T.T               # [N_V, N_V]  (symmetric!)
    M1     = eye + (1 - eye) * G                  # diagonal forced to 1
    A      = M1 * adj_v
    ret    = A @ (H_v @ weight) + bias            # [N_V, OUT_V]
    return (ret, H_e)

Distribution (8 NeuronCores, SPMD single NEFF), v4 — balanced symmetry:
    Core c owns output rows R_c = [512c, 512(c+1)).  G is symmetric, so only
    node-block pairs at ring distance 0..4 are computed: core c computes
    G[j-block (c+g)%8, R_c] for g in {0..3} in full, and HALF of the
    distance-4 block — the two ends of each distance-4 pair compute disjoint
    halves of the e-contraction (a per-core host-side permutation of the
    e axis keeps the SPMD graph uniform) and exchange partials.  Every core
    thus does exactly 4.5 block-equivalents of matmul work (36/64 of naive).

    Blocks at distance 1..3 plus the distance-4 partial are exchanged via one
    AllGather; each core picks the tiles it needs with a partition-id-derived
    dynamic DMA offset and DMA-transposes them (G[j,i] = G[i,j], also true
    per e-subset).  The output GEMM accumulates transposed
    (ret.T[o,i] += W2[j,o].T-stationary @ A.T[j,i]-moving, one PSUM bank,
    N=512 per step); ret.T is PE-transposed back at the end.

    All per-core block indexing is pre-rotated host-side (TTt4/TTH/ADJt/HVTR
    are in ring order (c+g)%8) so all 8 cores run one identical graph.  The
    "diagonal forced to 1" term is applied as ret[i] += adj_v[i,i] * W2[i]
    at the end (adj diagonal zeroed host-side).

    Operands fp16 (host-cast); accumulation fp32 in PSUM.
"""

import numpy as np

N_V, N_E = 4096, 16384
IN_V, OUT_V, IN_E = 128, 128, 64
NCORES = 8
RB = N_V // NCORES          # 512 output rows per core
ETB = 16                    # e superblocks (1024 edges each)
EL = 8                      # e chunks (128 edges) per superblock
E_T = ETB * EL              # 128 e chunks total
G_BLK = 8                   # node blocks of 512
JBL = 4                     # j chunks (128) per j block
TILE_ELEMS = JBL * 128 * 512  # elements per exchanged [512, 512] tile

_F16 = np.float16
_cache = {}


def _build():
    import concourse.mybir as mybir
    from concourse import bacc
    from concourse.masks import make_identity
    from concourse.tile import TileContext

    dt = mybir.dt
    f16, f32 = dt.float16, dt.float32

    nc = bacc.Bacc("TRN2", target_bir_lowering=False, debug=False,
                   num_devices=NCORES)

    # ------- DRAM parameters (host-pretiled, per-core ring-rotated) ------
    TTt4 = nc.dram_tensor("TTt4", [4, ETB, 128, EL * 512], f16,
                          kind="ExternalInput")
    TTH = nc.dram_tensor("TTH", [ETB // 2, 128, EL * 512], f16,
                         kind="ExternalInput")
    TTB = nc.dram_tensor("TTB", [ETB, 128, EL * 512], f16, kind="ExternalInput")
    ADJt = nc.dram_tensor("ADJt", [G_BLK, 128, JBL * 512], f16,
                          kind="ExternalInput")
    HVTR = nc.dram_tensor("HVTR", [IN_V, N_V], f16, kind="ExternalInput")
    HVTB = nc.dram_tensor("HVTB", [IN_V, RB], f16, kind="ExternalInput")
    WT = nc.dram_tensor("WT", [IN_V, OUT_V], f16, kind="ExternalInput")
    HET = nc.dram_tensor("HET", [IN_E, N_E], f16, kind="ExternalInput")
    PC = nc.dram_tensor("PC", [IN_E, 1], f16, kind="ExternalInput")
    ADIAB = nc.dram_tensor("ADIAB", [128, RB], f32, kind="ExternalInput")
    BIAC = nc.dram_tensor("BIAC", [128, 1], f32, kind="ExternalInput")
    OUT = nc.dram_tensor("out", [RB, OUT_V], f32, kind="ExternalOutput")

    with TileContext(nc) as tc:
        const = tc.alloc_tile_pool(name="const", bufs=1)
        spool = tc.alloc_tile_pool(name="spool", bufs=1)
        dramp = tc.alloc_tile_pool(name="dramp", bufs=1, space="DRAM")
        hetp = tc.alloc_tile_pool(name="hetp", bufs=1)
        stage = tc.alloc_tile_pool(name="stage", bufs=2)
        psum0 = tc.alloc_tile_pool(name="psum0", bufs=2, space="PSUM")

        # long-lived
        wt_sb = const.tile([IN_V, OUT_V], f16)
        hvtb_sb = const.tile([IN_V, RB], f16)
        adiab_sb = const.tile([128, RB], f32)
        biac_sb = const.tile([128, 1], f32)
        ident_sb = const.tile([128, 128], f32)
        w2_sb = const.tile([128, N_V], f16)          # H_v @ weight (ring order)
        s_sb = spool.tile([128, E_T * 512], f16)     # scaled T.T[:, R_c]
        # prologue-only (pool released before the main loop).  Order matters:
        # later pools reuse this range bottom-up, so put the tiles whose last
        # read happens earliest (het: d-matmuls) at the bottom where the tt
        # stream lands, and the latest-read (d: S-scaling) higher up.
        het_sb = hetp.tile([IN_E, N_E], f16)
        hvt_sb = hetp.tile([IN_V, N_V], f16)
        d_sb = hetp.tile([128, E_T], f32)
        pc_sb = hetp.tile([IN_E, 1], f16)

        contrib = dramp.tile([4, JBL, 128, 512], f16, name="contrib")
        agout1 = dramp.tile([NCORES, 2 * TILE_ELEMS], f16, name="agout1",
                            addr_space="Shared")
        agout2 = dramp.tile([NCORES, 2 * TILE_ELEMS], f16, name="agout2",
                            addr_space="Shared")
        exst = dramp.tile([4, TILE_ELEMS], f16, name="exst")

        # HET lands first, chunked, so the d -> S-scale -> main-loop chain
        # starts as early as possible (it is the kernel's serial prefix)
        nc.sync.dma_start(out=pc_sb[:], in_=PC[:])
        for etb in range(ETB):
            nc.sync.dma_start(out=het_sb[:, etb * EL * 128:(etb + 1) * EL * 128],
                              in_=HET[:, etb * EL * 128:(etb + 1) * EL * 128])
        nc.sync.dma_start(out=wt_sb[:], in_=WT[:])
        nc.sync.dma_start(out=hvt_sb[:], in_=HVTR[:])
        nc.sync.dma_start(out=hvtb_sb[:], in_=HVTB[:])
        nc.sync.dma_start(out=adiab_sb[:], in_=ADIAB[:])
        nc.sync.dma_start(out=biac_sb[:], in_=BIAC[:])
        make_identity(nc, ident_sb[:])

        # d[e] = (H_e @ p.T): one PSUM column per 128-edge chunk, copied out
        # per-superblock so S scaling (and the PE main loop) starts early
        d_ps = psum0.tile([128, E_T], f32, tag="dps")
        for etb in range(ETB):
            for el in range(EL):
                et = etb * EL + el
                nc.tensor.matmul(d_ps[:, et:et + 1],
                                 het_sb[:, et * 128:(et + 1) * 128], pc_sb[:],
                                 start=True, stop=True)
            nc.vector.tensor_copy(d_sb[:, etb * EL:(etb + 1) * EL],
                                  d_ps[:, etb * EL:(etb + 1) * EL])

        # S = T.T[:, R_c] * d[e]  (per-partition scale), resident in SBUF
        for etb in range(ETB):
            sraw = stage.tile([128, EL * 512], f16, tag="sraw")
            nc.sync.dma_start(out=sraw[:], in_=TTB[etb])
            for el in range(EL):
                et = etb * EL + el
                nc.vector.tensor_scalar_mul(
                    s_sb[:, et * 512:(et + 1) * 512],
                    sraw[:, el * 512:(el + 1) * 512],
                    d_sb[:, et:et + 1])

        # W2 = H_v @ weight in ring order ([j-partition, o] tiles)
        for jb in range(N_V // 128):
            w_ps = psum0.tile([128, OUT_V], f32, tag="wps")
            nc.tensor.matmul(w_ps[:], hvt_sb[:, jb * 128:(jb + 1) * 128],
                             wt_sb[:], start=True, stop=True)
            nc.vector.tensor_copy(w2_sb[:, jb * 128:(jb + 1) * 128], w_ps[:])

        psum0.release()
        stage.release()
        hetp.release()

        ttp = tc.alloc_tile_pool(name="ttp", bufs=3)
        adjp = tc.alloc_tile_pool(name="adjp", bufs=3)
        atp = tc.alloc_tile_pool(name="atp", bufs=4)
        gexp = tc.alloc_tile_pool(name="gexp", bufs=2)
        g4p = tc.alloc_tile_pool(name="g4p", bufs=1)
        rtp = tc.alloc_tile_pool(name="rtp", bufs=4)
        rxat = tc.alloc_tile_pool(name="rxat", bufs=1)
        outp = tc.alloc_tile_pool(name="outp", bufs=1)
        psum_m = tc.alloc_tile_pool(name="psum_m", bufs=5, space="PSUM")
        psum_r = tc.alloc_tile_pool(name="psum_r", bufs=1, space="PSUM")

        retT_ps = psum_r.tile([128, RB], f32, name="retT")  # [o, i], 1 bank
        n_ret = G_BLK * JBL
        ret_idx = 0

        def ret_mm(at, jb):
            # ret.T[o, i] += W2[jb-chunk, o].T @ A.T[jb-chunk, i]
            nonlocal ret_idx
            nc.tensor.matmul(retT_ps[:], w2_sb[:, jb * 128:(jb + 1) * 128],
                             at[:], start=(ret_idx == 0),
                             stop=(ret_idx == n_ret - 1))
            ret_idx += 1

        # own half of the distance-4 block, kept for the post-exchange sum
        gex4 = [g4p.tile([128, 512], f16, tag=f"g4_{j}", name=f"g4_{j}")
                for j in range(JBL)]

        # ---- main loop, part 1: blocks at ring distance 1,2,3 + d4-half ----
        def block(g):
            half = (g == 4)
            n_etb = ETB // 2 if half else ETB
            last_et = n_etb * EL - 1
            adj_g = None
            if not half:
                adj_g = adjp.tile([128, JBL * 512], f16, tag="adjg",
                                  name=f"adj_{g}")
                nc.scalar.dma_start(out=adj_g[:], in_=ADJt[g])
            m_ps = [psum_m.tile([128, 512], f32, tag="mps", name=f"m_{g}_{j}")
                    for j in range(JBL)]
            for etb in range(n_etb):
                tt = ttp.tile([128, EL * 512], f16, tag="tt",
                              name=f"tt_{g}_{etb}")
                nc.scalar.dma_start(out=tt[:],
                                     in_=TTH[etb] if half else TTt4[g, etb])
                for el in range(EL):
                    et = etb * EL + el
                    s_slice = s_sb[:, et * 512:(et + 1) * 512]
                    for jbl in range(JBL):
                        nc.tensor.matmul(
                            m_ps[jbl][:],
                            tt[:, el * 512 + jbl * 128:el * 512 + (jbl + 1) * 128],
                            s_slice,
                            start=(et == 0), stop=(et == last_et))
            for jbl in range(JBL):
                jb = g * JBL + jbl
                if half:
                    nc.vector.tensor_copy(gex4[jbl][:], m_ps[jbl][:])
                    nc.sync.dma_start(out=contrib[3, jbl], in_=gex4[jbl][:])
                    continue
                at = atp.tile([128, 512], f16, tag="at", name=f"at_{jb}")
                if g in (1, 2, 3):
                    gex = gexp.tile([128, 512], f16, tag="gex",
                                    name=f"gex_{jb}")
                    nc.vector.tensor_copy(gex[:], m_ps[jbl][:])
                    nc.sync.dma_start(out=contrib[g - 1, jbl], in_=gex[:])
                    nc.vector.tensor_mul(at[:], gex[:],
                                         adj_g[:, jbl * 512:(jbl + 1) * 512])
                else:
                    nc.vector.tensor_mul(at[:], m_ps[jbl][:],
                                         adj_g[:, jbl * 512:(jbl + 1) * 512])
                ret_mm(at, jb)

        rts = {}
        exr = exst[:].rearrange("a (r c) -> a r c", r=512)

        def recv_prep(d, ag, ag_slot):
            # gather the needed peer tile (dynamic rank offset), transpose it
            # (G[j,i] = G[i,j]), blend with adj; consumed by ret_mm later
            srcr = nc.gpsimd.alloc_register(f"src{d}")
            pid = nc.gpsimd.partition_id()
            nc.gpsimd.reg_add(srcr, pid, d)
            nc.gpsimd.reg_mod(srcr, srcr, NCORES)
            srcv = nc.gpsimd.snap(srcr, donate=True, min_val=0,
                                  max_val=NCORES - 1)
            nc.gpsimd.dma_start(
                out=exst[d - 4:d - 3, :],
                in_=ag[_ds(srcv, 1),
                       ag_slot * TILE_ELEMS:(ag_slot + 1) * TILE_ELEMS])
            adj_g = adjp.tile([128, JBL * 512], f16, tag="adjg",
                              name=f"adj_{d}")
            nc.sync.dma_start(out=adj_g[:], in_=ADJt[d])
            for jbl in range(JBL):
                jb = d * JBL + jbl
                rt = rtp.tile([128, 512], f16, tag="rt", name=f"rt_{jb}")
                nc.sync.dma_start_transpose(
                    rt[:], exr[d - 4, :, jbl * 128:(jbl + 1) * 128])
                at = rxat.tile([128, 512], f16, tag=f"rx_{jbl + (d % 2) * 4}",
                               name=f"rx_{jb}")
                if d == 4:
                    nc.vector.tensor_add(at[:], rt[:], gex4[jbl][:])
                    nc.vector.tensor_mul(at[:], at[:],
                                         adj_g[:, jbl * 512:(jbl + 1) * 512])
                else:
                    nc.vector.tensor_mul(at[:], rt[:],
                                         adj_g[:, jbl * 512:(jbl + 1) * 512])
                rts[jb] = at

        # Two AllGathers: the first doubles as a cross-core barrier that
        # absorbs start-time skew while plenty of compute remains; the second
        # then completes quickly and hides under the distance-0 block.
        block(1)
        block(2)
        nc.gpsimd.collective_compute(
            "AllGather", mybir.AluOpType.bypass,
            replica_groups=[list(range(NCORES))],
            ins=[contrib[0:2].opt()],
            outs=[agout1[:].opt()])
        recv_prep(7, agout1, 0)
        recv_prep(6, agout1, 1)
        block(3)
        block(4)
        nc.gpsimd.collective_compute(
            "AllGather", mybir.AluOpType.bypass,
            replica_groups=[list(range(NCORES))],
            ins=[contrib[2:4].opt()],
            outs=[agout2[:].opt()])
        recv_prep(5, agout2, 0)
        recv_prep(4, agout2, 1)

        # distance 6/7 tiles are long since ready -> fold in before block 0
        for d in (7, 6):
            for jbl in range(JBL):
                ret_mm(rts[d * JBL + jbl], d * JBL + jbl)

        # ---- main loop, part 2: the local distance-0 block ----
        block(0)

        for d in (5, 4):
            for jbl in range(JBL):
                ret_mm(rts[d * JBL + jbl], d * JBL + jbl)
        assert ret_idx == n_ret

        # ---- epilogue ----
        psum_e = tc.alloc_tile_pool(name="psum_e", bufs=1, space="PSUM")
        # W2[R_c].T[o, i] for the diagonal correction
        w2bt_ps = psum_e.tile([128, RB], f32, tag="w2bt")
        nc.tensor.matmul(w2bt_ps[:], wt_sb[:], hvtb_sb[:],
                         start=True, stop=True)
        corr = outp.tile([128, RB], f32, tag="corr", name="corr")
        nc.vector.tensor_mul(corr[:], w2bt_ps[:], adiab_sb[:])
        sum1 = outp.tile([128, RB], f32, tag="sum1", name="sum1")
        nc.vector.tensor_add(sum1[:], retT_ps[:], corr[:])
        res = outp.tile([128, RB], f32, tag="res", name="res")
        nc.vector.tensor_scalar_add(res[:], sum1[:], biac_sb[:, 0:1])
        # transpose [o, i] -> [i, o] and store
        for ic in range(RB // 128):
            t_ps = psum_e.tile([128, 128], f32, tag="tps", name=f"t_{ic}")
            nc.tensor.transpose(t_ps[:], res[:, ic * 128:(ic + 1) * 128],
                                ident_sb[:])
            o_sb = outp.tile([128, OUT_V], f32, tag="osb", name=f"o_{ic}")
            nc.vector.tensor_copy(o_sb[:], t_ps[:])
            nc.sync.dma_start(out=OUT[ic * 128:(ic + 1) * 128, :], in_=o_sb[:])

        psum_e.release()
        psum_r.release()
        psum_m.release()
        outp.release()
        rxat.release()
        rtp.release()
        g4p.release()
        gexp.release()
        atp.release()
        adjp.release()
        ttp.release()
        dramp.release()
        spool.release()
        const.release()

    nc.compile()
    return nc


def _ds(start, size):
    import concourse.bass as bass
    return bass.ds(start, size)


def _prep_inputs(H_v, H_e, adj_v, T, weight, p, bias):
    """Host-side shard/retile/rotate/e-permute. Returns in_maps for 8 cores."""
    TT16 = np.ascontiguousarray(T.T).astype(_F16)          # [N_E, N_V]
    A = TT16.reshape(ETB, EL, 128, G_BLK, 512)
    TTt_all = np.ascontiguousarray(A.transpose(3, 0, 2, 1, 4)).reshape(
        G_BLK, ETB, 128, EL * 512)                          # [b, etb, p, el*j]

    HVT = np.ascontiguousarray(H_v.T).astype(_F16)         # [IN_V, N_V]
    WT = weight.astype(_F16)
    HET_n = np.ascontiguousarray(H_e.T).astype(_F16)       # [IN_E, N_E]
    PCm = np.ascontiguousarray(p.T).astype(_F16)           # [IN_E, 1]
    BIAC = np.ascontiguousarray(bias.astype(np.float32)[:, None])
    adjT = adj_v.T                                          # [j, i] view
    diag = np.ascontiguousarray(np.diag(adj_v)).astype(np.float32)

    in_maps = []
    for c in range(NCORES):
        r0, r1 = c * RB, (c + 1) * RB
        ring = [(c + g) % G_BLK for g in range(G_BLK)]
        if c < NCORES // 2:
            eperm = list(range(ETB))
        else:
            eperm = list(range(ETB // 2, ETB)) + list(range(ETB // 2))
        TTb = TT16[:, r0:r1]                                # [N_E, RB]
        B = TTb.reshape(ETB, EL, 128, 512)
        TTB = np.ascontiguousarray(
            B.transpose(0, 2, 1, 3)[eperm]).reshape(ETB, 128, EL * 512)
        HET = np.ascontiguousarray(
            HET_n.reshape(IN_E, ETB, EL * 128)[:, eperm]).reshape(IN_E, N_E)
        adjblk = adjT[:, r0:r1].astype(_F16)                # [N_V, RB]
        adjblk[r0 + np.arange(RB), np.arange(RB)] = 0       # zero the diagonal
        C = adjblk.reshape(G_BLK, JBL, 128, 512).transpose(0, 2, 1, 3)
        ADJt = np.ascontiguousarray(C[ring]).reshape(G_BLK, 128, JBL * 512)
        cols = np.concatenate([np.arange(b * RB, (b + 1) * RB) for b in ring])
        HVTR = np.ascontiguousarray(HVT[:, cols])
        ADIAB = np.ascontiguousarray(
            np.broadcast_to(diag[r0:r1], (128, RB)))
        ttg = TTt_all[:, eperm]                             # e-permuted blocks
        in_maps.append({
            "TTt4": np.ascontiguousarray(ttg[ring[:4]]),
            "TTH": np.ascontiguousarray(ttg[ring[4], :ETB // 2]),
            "TTB": TTB,
            "ADJt": ADJt,
            "HVTR": HVTR,
            "HVTB": np.ascontiguousarray(HVT[:, r0:r1]),
            "WT": WT,
            "HET": HET,
            "PC": PCm,
            "ADIAB": ADIAB,
            "BIAC": BIAC,
        })
    return in_maps


def kernel(H_v, H_e, adj_e, adj_v, T, weight, p, bias):
    from concourse.bass_utils import run_bass_kernel_spmd

    H_v = np.asarray(H_v, dtype=np.float32)
    H_e = np.asarray(H_e, dtype=np.float32)
    adj_v = np.asarray(adj_v, dtype=np.float32)
    T = np.asarray(T, dtype=np.float32)
    weight = np.asarray(weight, dtype=np.float32)
    p = np.asarray(p, dtype=np.float32)
    bias = np.asarray(bias, dtype=np.float32)

    if "nc" not in _cache:
        _cache["nc"] = _build()
    nc = _cache["nc"]

    in_maps = _prep_inputs(H_v, H_e, adj_v, T, weight, p, bias)
    res = run_bass_kernel_spmd(nc, in_maps, list(range(NCORES)))
    ret = np.concatenate([res.results[c]["out"] for c in range(NCORES)], axis=0)
    return (ret, H_e)
